# revision 6
# baseline (speedup 1.0000x reference)
"""LCGP prediction kernel for Trainium2, sharded over 8 NeuronCores.

Strategy (expert-parallel over the q=8 GP components, one per core):
  Per core q, split the n0=2048 test axis into 2 halves of mh=1024:
    phase 1: C0T[n, m] = exp(lLmb0[q] + ln(S) - ||a_m - b_n||^2) computed by a
        fused PE matmul over hi/lo-split fp16 feature rows; ACT exp emits the
        scaled C0T in fp16 (c0t16), DVE down-converts to fp8e4 (c0t8).
    ghat[m]  = C0T.T @ CinvM[q] in fp16 (precision-critical path).
    phase 2: t = C0 @ Th[q] as an fp8e4 DoubleRow GEMM (256-deep contraction
        per matmul, 0.5 cyc/col): Th is pre-scaled/converted to fp8 on host
        and streamed once per half. sumt2[m] = sum_r t[m,r]^2 via ACT Square
        with accum_out on each [128,512] PSUM tile.
  Host: tiny [q,n0] -> [p,n0] psi projection in fp32 numpy.

fp8 quantization error analysis: sumt2 averages quantization noise over the
r-contraction (rel err ~1e-3); ghat stays fp16 end-to-end (~1e-4).
"""

import os

import numpy as np
import ml_dtypes

import concourse.bacc as bacc
import concourse.bass as bass
import concourse.mybir as mybir
import concourse.tile as tile

P = 128
FP32 = mybir.dt.float32
FP16 = mybir.dt.float16
FP8 = mybir.dt.float8e4
F8NP = ml_dtypes.float8_e4m3

# Full-size problem dims (hardcoded per spec: q=8, d=8, p=64, n=4096, n0=2048)
Q_FULL = 8
N_FULL = 4096
N0_FULL = 2048

S_C0 = np.float32(32.0)          # C0 pre-scale folded into the exp bias
LN_S_C0 = float(np.log(S_C0))


def build_nc(n=N_FULL, n0=N0_FULL, rb=512, mh=1024, fk=32, mc=512, debug=False):
    """Build the single-core Bass program (same program on all 8 cores)."""
    kt = n // P            # 32 contraction k-tiles of 128
    kt2 = kt // 2          # 16 DoubleRow k-steps of 256
    nrb = n // rb          # 8 r-blocks of the big GEMM
    nh = n0 // mh          # 2 m-halves
    mt = mh // P           # 8 m-tiles per half
    nmc = mh // mc         # 2 phase-1 chunks per half

    nc = bacc.Bacc("TRN2", target_bir_lowering=False, debug=debug)

    a_feat = nc.dram_tensor("a_feat", [fk, n0], FP16, kind="ExternalInput")
    b_feat = nc.dram_tensor("b_feat", [fk, n], FP16, kind="ExternalInput")
    th8 = nc.dram_tensor("th8", [P, kt2, 2, n], FP8, kind="ExternalInput")
    cinv = nc.dram_tensor("cinv", [P, kt], FP16, kind="ExternalInput")
    ghat_o = nc.dram_tensor("ghat", [n0 // P, P], FP32, kind="ExternalOutput")
    # raw per-r-block square sums; host reduces the last axis
    sumt2_o = nc.dram_tensor("sumt2", [n0 // P, P, nrb], FP32,
                             kind="ExternalOutput")

    with tile.TileContext(nc) as tc:
        with (
            tc.tile_pool(name="feat", bufs=1) as featp,
            tc.tile_pool(name="c16", bufs=1) as c16p,
            tc.tile_pool(name="c8", bufs=2) as c8p,
            tc.tile_pool(name="slab", bufs=2 * kt2) as slabp,
            tc.tile_pool(name="scr", bufs=4) as scrp,
            tc.tile_pool(name="gsb", bufs=2 * mt + 4) as gsbp,
            tc.tile_pool(name="sqps", bufs=4, space=bass.MemorySpace.PSUM) as sqpsp,
            tc.tile_pool(name="tps", bufs=3, space=bass.MemorySpace.PSUM) as tpsp,
            tc.tile_pool(name="gps", bufs=1, space=bass.MemorySpace.PSUM) as gpsp,
        ):
            bf = featp.tile([fk, n], FP16, tag="bf")
            af = featp.tile([fk, n0], FP16, tag="af")
            cv = featp.tile([P, kt], FP16, tag="cv")
            # fine-grained input DMAs so the first phase-1 matmul starts early
            for o in range(0, n, mc):
                nc.sync.dma_start(bf[:, o:o + mc], b_feat[:, o:o + mc])
            for o in range(0, n0, mc):
                nc.sync.dma_start(af[:, o:o + mc], a_feat[:, o:o + mc])
            nc.sync.dma_start(cv[:], cinv[:])

            c16s, c8s = [], []
            for h in range(nh):
                # ---- phase 1: scaled C0T tiles (fp16 + fp8) for this half ----
                c16 = c16p.tile([P, kt, mh], FP16, tag="c16", name=f"c16_{h}")
                c8 = c8p.tile([P, kt, mh], FP8, tag="c8", name=f"c8_{h}")
                c16s.append(c16)
                c8s.append(c8)
                for j in range(kt):
                    for c in range(nmc):
                        ps = sqpsp.tile([P, mc], FP32, tag="sqps")
                        nc.tensor.matmul(
                            ps[:],
                            bf[:, j * P:(j + 1) * P],
                            af[:, h * mh + c * mc: h * mh + (c + 1) * mc],
                            start=True, stop=True,
                        )
                        nc.scalar.activation(
                            c16[:, j, c * mc:(c + 1) * mc], ps[:],
                            mybir.ActivationFunctionType.Exp,
                            bias=0.0, scale=-1.0,
                        )
                        nc.vector.tensor_copy(
                            c8[:, j, c * mc:(c + 1) * mc],
                            c16[:, j, c * mc:(c + 1) * mc],
                        )

                # ---- ghat (fp16, precision-critical) ----
                for i in range(mt):
                    gp = gpsp.tile([P, 1], FP32, tag="gps", name=f"gp_{h}_{i}")
                    for j in range(kt):
                        nc.tensor.matmul(
                            gp[:], c16[:, j, i * P:(i + 1) * P], cv[:, j:j + 1],
                            start=(j == 0), stop=(j == kt - 1),
                            skip_group_check=True,
                        )
                    gh = gsbp.tile([P, 1], FP32, tag="ghsb")
                    nc.vector.tensor_copy(gh[:], gp[:])
                    nc.sync.dma_start(ghat_o[h * mt + i, :], gh[:])

            # ---- phase 2: fp8 DoubleRow GEMM over r-blocks, both halves ----
            for h in range(nh):
                c8 = c8s[h]
                gaccs = [gsbp.tile([P, nrb], FP32, tag="gacc",
                                   name=f"gacc_{h}_{i}")
                         for i in range(mt)]
                for r in range(nrb):
                    slabs = []
                    for kk in range(kt2):
                        sl = slabp.tile([P, 2, rb], FP8, tag="slab")
                        nc.sync.dma_start(
                            sl[:], th8[:, kk, :, r * rb:(r + 1) * rb])
                        slabs.append(sl)
                    for i in range(mt):
                        tp = tpsp.tile([P, rb], FP32, tag="tps")
                        for kk in range(kt2):
                            nc.tensor.matmul(
                                tp[:],
                                c8[:, 2 * kk:2 * kk + 2, i * P:(i + 1) * P],
                                slabs[kk][:],
                                start=(kk == 0), stop=(kk == kt2 - 1),
                                perf_mode=mybir.MatmulPerfMode.DoubleRow,
                                skip_group_check=True,
                            )
                        sc = scrp.tile([P, rb], FP16, tag="scr",
                                       name=f"sc_{h}_{r}_{i}")
                        nc.scalar.activation(
                            sc[:], tp[:], mybir.ActivationFunctionType.Square,
                            accum_out=gaccs[i][:, r:r + 1],
                        )
                for i in range(mt):
                    nc.sync.dma_start(sumt2_o[h * mt + i], gaccs[i][:])

    nc.compile()
    return nc


def _features_for_q(x0s, x, inv_l_q, lLmb0_q, fk=32):
    """Host prep: hi/lo-split fp16 feature rows so the PE computes
    sq_mod[n, m] = ||a_m - b_n||^2 - lLmb0 - ln(S_C0) in near-fp32 precision."""
    f16, f32 = np.float16, np.float32
    a = (x0s * inv_l_q).astype(f32)            # [n0, d]
    b = (x * inv_l_q).astype(f32)              # [n, d]
    sqa = (a * a).sum(-1, dtype=f32) - f32(lLmb0_q) - f32(LN_S_C0)
    sqb = (b * b).sum(-1, dtype=f32)

    def hilo(v):
        hi = v.astype(f16)
        lo = (v - hi.astype(f32)).astype(f16)
        return hi, lo

    a_hi, a_lo = hilo(a)
    b_hi, b_lo = hilo(b)
    sqa_hi, sqa_lo = hilo(sqa)
    sqb_hi, sqb_lo = hilo(sqb)
    d = a.shape[1]
    n0, n = a.shape[0], b.shape[0]
    assert 3 * d + 4 <= fk
    af = np.zeros((fk, n0), f16)
    bf = np.zeros((fk, n), f16)
    m2a_hi = (-2.0 * a_hi.astype(f32)).astype(f16).T   # exact in fp16
    m2a_lo = (-2.0 * a_lo.astype(f32)).astype(f16).T
    af[0:d] = m2a_hi
    af[d:2 * d] = m2a_hi
    af[2 * d:3 * d] = m2a_lo
    af[3 * d] = sqa_hi
    af[3 * d + 1] = sqa_lo
    af[3 * d + 2] = 1.0
    af[3 * d + 3] = 1.0
    bf[0:d] = b_hi.T
    bf[d:2 * d] = b_lo.T
    bf[2 * d:3 * d] = b_hi.T
    bf[3 * d] = 1.0
    bf[3 * d + 1] = 1.0
    bf[3 * d + 2] = sqb_hi
    bf[3 * d + 3] = sqb_lo
    return af, bf


def _th_scale(th_q):
    """Power-of-two scale putting max |Th| into (96, 192] for fp8e4."""
    m = float(np.abs(th_q).max())
    if m == 0.0:
        return np.float32(1.0)
    return np.float32(2.0 ** np.floor(np.log2(192.0 / m)))


def prep_core_inputs(inputs, q, fk=32):
    """Per-core (per-component) input map for the device kernel."""
    f16, f32 = np.float16, np.float32
    x0 = np.asarray(inputs["x0"], f32)
    x = np.asarray(inputs["x"], f32)
    x_min = np.asarray(inputs["x_min"], f32)
    x_max = np.asarray(inputs["x_max"], f32)
    lLmb = np.asarray(inputs["lLmb"], f32)
    lLmb0 = np.asarray(inputs["lLmb0"], f32)
    x0s = (x0 - x_min) / (x_max - x_min)
    inv_l = np.exp(-0.5 * lLmb[q]).astype(f32)
    af, bf = _features_for_q(x0s, x, inv_l, lLmb0[q], fk=fk)
    cinv = np.asarray(inputs["CinvM"], f32)[q].astype(f16)
    n = cinv.shape[0]
    cinv_t = np.ascontiguousarray(cinv.reshape(n // P, P).T)   # [128, kt]
    th_q = np.asarray(inputs["Th"], f32)[q]
    s_th = _th_scale(th_q)
    # [128, kt2, 2, n]: th8[p, kk, i, c] = s_th * Th[kk*256 + i*128 + p, c]
    th8 = np.ascontiguousarray(
        (th_q.reshape(n // 256, 2, P, n) * s_th).transpose(2, 0, 1, 3)
    ).astype(F8NP)
    return {"a_feat": af, "b_feat": bf, "th8": th8, "cinv": cinv_t}


def finish_host(inputs, ghat_all, sumt2_all):
    """Final tiny [q,n0] -> [p,n0] projection, fp32 on host (mirrors reference)."""
    f32 = np.float32
    lLmb0 = np.asarray(inputs["lLmb0"], f32)
    lnug = np.asarray(inputs["lnugGPs"], f32)
    lsig = np.asarray(inputs["lsigma2s"], f32)
    phi = np.asarray(inputs["phi"], f32)
    ystd = np.asarray(inputs["ystd"], f32)
    ymean = np.asarray(inputs["ymean"], f32)

    c00 = (np.exp(lLmb0) * (1.0 + np.exp(lnug))).astype(f32)[:, None]
    gvar = c00 - sumt2_all                        # [q, n0]
    sig = np.exp(lsig).astype(f32)                # [p]
    psi = (phi * np.sqrt(sig)[:, None]).astype(f32)
    predmean = (psi @ ghat_all).astype(f32)       # [p, n0]
    confvar = (gvar.T @ (psi ** 2).T).astype(f32)  # [n0, p]
    predvar = confvar + sig
    ypred = (predmean * ystd + ymean).astype(f32)
    yconfvar = (confvar.T * ystd ** 2).astype(f32)
    ypredvar = (predvar.T * ystd ** 2).astype(f32)
    return ypred, ypredvar, yconfvar


_NC_CACHE = {}
LAST_RESULTS = None


def kernel(**inputs):
    from concourse.bass_utils import run_bass_kernel_spmd

    global LAST_RESULTS
    q_n = Q_FULL
    n0 = N0_FULL

    if "nc" not in _NC_CACHE:
        _NC_CACHE["nc"] = build_nc()
    nc = _NC_CACHE["nc"]

    th_f32 = np.asarray(inputs["Th"], np.float32)
    s_ths = [_th_scale(th_f32[q]) for q in range(q_n)]
    in_maps = [prep_core_inputs(inputs, q) for q in range(q_n)]
    core_ids = list(range(q_n))
    res = run_bass_kernel_spmd(
        nc, in_maps, core_ids,
        trace=bool(os.environ.get("LCGP_TRACE")),
    )
    LAST_RESULTS = res

    ghat_all = np.zeros((q_n, n0), np.float32)
    sumt2_all = np.zeros((q_n, n0), np.float32)
    for q in range(q_n):
        ghat_all[q] = np.asarray(res.results[q]["ghat"]).reshape(n0) / S_C0
        raw = np.asarray(res.results[q]["sumt2"], np.float32)  # [16, 128, nrb]
        sumt2_all[q] = raw.sum(-1, dtype=np.float32).reshape(n0) \
            / (S_C0 * s_ths[q]) ** 2

    return finish_host(inputs, ghat_all, sumt2_all)
